# revision 11
# baseline (speedup 1.0000x reference)
"""Multi-head attention (B=4, S=2048, D=1024, H=16, dh=64, causal) on 8
Trainium2 NeuronCores.

Sharding: core (b, g) = batch b in 0..3, head-group g in 0..1 (8 heads each).
Each core computes attention for its 8 heads over its batch and a partial
output projection; the host sums the two head-group partials per batch and
adds the bias corrections (bo + bv @ Wo, since the V bias is not applied on
device -- attn rows sum to 1 so it folds into a constant row).
"""

import sys
import types

import numpy as np


def _install_ntff_shim():
    """antenv.axon_hooks is absent in this image; recreate it and register the
    ctypes NTFF profile hook like trn_boot would, so trace=True works."""
    if "antenv.axon_hooks" in sys.modules:
        return
    mod = types.ModuleType("antenv.axon_hooks")
    state = {"hook": None}
    mod.set_axon_ntff_profile_hook = lambda h: state.__setitem__("hook", h)
    mod.get_axon_ntff_profile_hook = lambda: state["hook"]
    sys.modules["antenv.axon_hooks"] = mod
    try:
        import antenv

        antenv.axon_hooks = mod
    except ImportError:
        pass
    try:
        from trn_agent_boot.trn_boot import _ntff_profile_via_ctypes

        mod.set_axon_ntff_profile_hook(
            _ntff_profile_via_ctypes("/opt/axon/libaxon_pjrt.so")
        )
    except Exception:
        pass


_install_ntff_shim()

import concourse.bacc as bacc  # noqa: E402
import concourse.mybir as mybir  # noqa: E402
import concourse.tile as tile  # noqa: E402

P = 128
D = 1024
FG = 512  # features per core = 8 heads x 64
NH = 8  # heads per core
DH = 64
KC = D // P  # 8 contraction chunks for the projections
FC = FG // P  # 4 feature chunks of 128
TQ = 512  # q tile (free dim)
TK = 128  # k tile (partition dim)
F32 = mybir.dt.float32
F32R = mybir.dt.float32r
AF = mybir.ActivationFunctionType


def build(tokens=2048, causal=True):
    S = tokens
    NQC = S // TQ
    NKC = S // TK
    nc = bacc.Bacc()
    xt = nc.dram_tensor("XT", [D, S], F32R, kind="ExternalInput")
    wq = nc.dram_tensor("WQ", [D, FG], F32R, kind="ExternalInput")
    wk = nc.dram_tensor("WK", [D, FG], F32R, kind="ExternalInput")
    wv = nc.dram_tensor("WV", [D, FG], F32R, kind="ExternalInput")
    bq = nc.dram_tensor("BQ", [P, FC], F32, kind="ExternalInput")
    bk = nc.dram_tensor("BK", [P, FC], F32, kind="ExternalInput")
    wo = nc.dram_tensor("WO", [P, FC, D], F32R, kind="ExternalInput")
    cm = nc.dram_tensor("CM", [P, 4, TQ], F32R, kind="ExternalInput")
    one_d = nc.dram_tensor("ONE", [1, DH], F32R, kind="ExternalInput")
    vone_d = nc.dram_tensor(
        "VONES", [P, S // TK, NH, 1], F32R, kind="ExternalInput"
    )
    out = nc.dram_tensor("OUT", [S, D], F32, kind="ExternalOutput")

    with tile.TileContext(nc) as tc, nc.allow_low_precision(
        reason="float32r matmul inputs"
    ):
        with tc.tile_pool(name="const", bufs=1) as cpool, tc.tile_pool(
            name="qkv", bufs=1
        ) as qkv:
            cm_sb = cpool.tile([P, 4, TQ], F32R, name="cm_sb")
            nc.sync.dma_start(cm_sb[:], cm[:])
            bq_sb = cpool.tile([P, FC], F32, name="bq_sb")
            nc.sync.dma_start(bq_sb[:], bq[:])
            bk_sb = cpool.tile([P, FC], F32, name="bk_sb")
            nc.sync.dma_start(bk_sb[:], bk[:])
            one_sb = cpool.tile([1, DH], F32R, name="one_sb")
            nc.sync.dma_start(one_sb[:], one_d[:])

            qt_sb = qkv.tile([P, FC, S], F32R, name="qt_sb")
            kt_sb = qkv.tile([P, FC, S], F32R, name="kt_sb")
            v_sb = qkv.tile([P, NKC, NH, DH + 1], F32R, name="v_sb")
            nc.sync.dma_start(v_sb[:, :, :, DH : DH + 1], vone_d[:])

            # ---- projections: Q^T, K^T (feature-major), V (token-major) ----
            with tc.tile_pool(name="w", bufs=1) as wpool, tc.tile_pool(
                name="xt", bufs=2
            ) as xpool, tc.tile_pool(name="psproj", bufs=4, space="PSUM") as pj:
                wq_sb = wpool.tile([P, KC, FG], F32R, name="wq_sb")
                nc.sync.dma_start(
                    wq_sb[:], wq.rearrange("(kc p) m -> p kc m", p=P)
                )
                wk_sb = wpool.tile([P, KC, FG], F32R, name="wk_sb")
                nc.sync.dma_start(
                    wk_sb[:], wk.rearrange("(kc p) m -> p kc m", p=P)
                )
                wv_sb = wpool.tile([P, KC, FG], F32R, name="wv_sb")
                nc.sync.dma_start(
                    wv_sb[:], wv.rearrange("(kc p) m -> p kc m", p=P)
                )
                for q4 in range(NQC):
                    tsl = slice(q4 * TQ, (q4 + 1) * TQ)
                    xt_t = xpool.tile([P, KC, TQ], F32R, tag="xt", name="xt_t")
                    nc.sync.dma_start(
                        xt_t[:],
                        xt[:, tsl].rearrange("(kc p) t -> p kc t", p=P),
                    )
                    for w_sb, b_sb, dst in (
                        (wq_sb, bq_sb, qt_sb),
                        (wk_sb, bk_sb, kt_sb),
                    ):
                        for fc in range(FC):
                            ps = pj.tile([P, TQ], F32, tag="pj", name="ps_pj")
                            for kc in range(KC):
                                nc.tensor.matmul(
                                    ps[:],
                                    w_sb[:, kc, fc * P : (fc + 1) * P],
                                    xt_t[:, kc, :],
                                    start=(kc == 0),
                                    stop=(kc == KC - 1),
                                )
                            nc.scalar.activation(
                                dst[:, fc, tsl],
                                ps[:],
                                AF.Identity,
                                bias=b_sb[:, fc : fc + 1],
                            )
                    for t4 in range(TQ // P):
                        ps = pj.tile([P, FG], F32, tag="pj", name="ps_pv")
                        for kc in range(KC):
                            nc.tensor.matmul(
                                ps[:],
                                xt_t[:, kc, t4 * P : (t4 + 1) * P],
                                wv_sb[:, kc, :],
                                start=(kc == 0),
                                stop=(kc == KC - 1),
                            )
                        tc_idx = q4 * (TQ // P) + t4
                        for h in range(NH):
                            nc.vector.tensor_copy(
                                v_sb[:, tc_idx, h, 0:DH],
                                ps[:, h * DH : (h + 1) * DH],
                            )

            # ---- attention ----
            with tc.tile_pool(name="wo", bufs=1) as wopool, tc.tile_pool(
                name="e", bufs=6
            ) as epool, tc.tile_pool(
                name="pss", bufs=2, space="PSUM"
            ) as pss, tc.tile_pool(
                name="pso", bufs=2, space="PSUM"
            ) as pso, tc.tile_pool(
                name="psb", bufs=2, space="PSUM"
            ) as psb, tc.tile_pool(name="u", bufs=1) as upool:
                wo_sb = wopool.tile([P, FC, D], F32R, name="wo_sb")
                nc.sync.dma_start(wo_sb[:], wo[:])
                u_sb = upool.tile([P, FC, S], F32R, name="u_sb")
                for h in range(NH):
                    hi, hp = h // 2, h % 2
                    qsl = slice(hp * DH, hp * DH + DH)
                    for qc in range(NQC):
                        nkc = min(NKC, 4 * (qc + 1)) if causal else NKC
                        qtsl = slice(qc * TQ, (qc + 1) * TQ)
                        po = pso.tile([DH + 1, TQ], F32, tag="po", name="po")
                        for kc in range(nkc):
                            ps = pss.tile([P, TQ], F32, tag="ps", name="ps_s")
                            nc.tensor.matmul(
                                ps[:],
                                kt_sb[qsl, hi, kc * TK : (kc + 1) * TK],
                                qt_sb[qsl, hi, qtsl],
                                start=True,
                                stop=True,
                            )
                            e_t = epool.tile([P, TQ], F32R, tag="e", name="e_t")
                            nc.scalar.activation(
                                e_t[:], ps[:], AF.Exp, scale=0.125
                            )
                            if causal and kc >= 4 * qc:
                                nc.vector.tensor_mul(
                                    e_t[:], e_t[:], cm_sb[:, kc - 4 * qc, :]
                                )
                            nc.tensor.matmul(
                                po[:],
                                v_sb[:, kc, h, :],
                                e_t[:],
                                start=(kc == 0),
                                stop=(kc == nkc - 1),
                                skip_group_check=True,
                            )
                        # normalize: recip of Z row, broadcast over 64
                        # partitions via K=1 matmul, multiply during eviction
                        rz_t = epool.tile(
                            [1, TQ], F32R, tag="rz", name="rz_t"
                        )
                        nc.vector.reciprocal(rz_t[:], po[DH : DH + 1, :])
                        pb = psb.tile([DH, TQ], F32, tag="pb", name="pb")
                        nc.tensor.matmul(
                            pb[:], one_sb[:], rz_t[:],
                            start=True, stop=True,
                        )
                        rzb_t = epool.tile(
                            [DH, TQ], F32, tag="rzb", name="rzb_t"
                        )
                        nc.vector.tensor_copy(rzb_t[:], pb[:])
                        nc.vector.tensor_mul(
                            u_sb[qsl, hi, qtsl], po[0:DH, :], rzb_t[:]
                        )

                # ---- output projection ----
                with tc.tile_pool(name="o", bufs=3) as opool, tc.tile_pool(
                    name="psj", bufs=2, space="PSUM"
                ) as psj:
                    for t8 in range(S // P):
                        o_t = opool.tile([P, D], F32, tag="o", name="o_t")
                        for n2 in range(2):
                            ps = psj.tile(
                                [P, 512], F32, tag="psj", name="ps_o"
                            )
                            for i in range(FC):
                                nc.tensor.matmul(
                                    ps[:],
                                    u_sb[:, i, t8 * P : (t8 + 1) * P],
                                    wo_sb[:, i, n2 * 512 : (n2 + 1) * 512],
                                    start=(i == 0),
                                    stop=(i == FC - 1),
                                )
                            nc.vector.tensor_copy(
                                o_t[:, n2 * 512 : (n2 + 1) * 512], ps[:]
                            )
                        nc.sync.dma_start(out[t8 * P : (t8 + 1) * P, :], o_t[:])

    nc.compile()
    return nc


def make_masks():
    """mask[p, j, f] = 1.0 where k-position p of diagonal chunk j is allowed
    to attend for q-position f of the 512-wide q tile: p <= f - 128*j."""
    p = np.arange(P)[:, None, None]
    j = np.arange(4)[None, :, None]
    f = np.arange(TQ)[None, None, :]
    return (p <= f - TK * j).astype(np.float32)


def make_in_maps(X, Wq, bq, Wk, bk, Wv, Wo, causal):
    cmv = make_masks() if causal else np.zeros((P, 4, TQ), np.float32)
    in_maps = []
    for b in range(4):
        for g in range(2):
            sl = slice(g * FG, (g + 1) * FG)
            in_maps.append(
                {
                    "XT": np.ascontiguousarray(X[b].T),
                    "WQ": np.ascontiguousarray(Wq[:, sl]),
                    "WK": np.ascontiguousarray(Wk[:, sl]),
                    "WV": np.ascontiguousarray(Wv[:, sl]),
                    "BQ": np.ascontiguousarray(bq[sl].reshape(FC, P).T),
                    "BK": np.ascontiguousarray(bk[sl].reshape(FC, P).T),
                    "WO": np.ascontiguousarray(
                        Wo[sl, :].reshape(FC, P, D).transpose(1, 0, 2)
                    ),
                    "CM": cmv,
                    "ONE": np.ones((1, DH), np.float32),
                    "VONES": np.ones((P, X.shape[1] // TK, NH, 1), np.float32),
                }
            )
    return in_maps


_CACHE = {}


def _get_program(causal):
    key = bool(causal)
    if key not in _CACHE:
        _CACHE[key] = build(tokens=2048, causal=key)
    return _CACHE[key]


def kernel(X, Wq, bq, Wk, bk, Wv, bv, Wo, bo, causal, **_unused):
    from concourse.bass_utils import run_bass_kernel_spmd

    X = np.asarray(X, np.float32)
    Wq, bq = np.asarray(Wq, np.float32), np.asarray(bq, np.float32)
    Wk, bk = np.asarray(Wk, np.float32), np.asarray(bk, np.float32)
    Wv = np.asarray(Wv, np.float32)
    Wo, bo = np.asarray(Wo, np.float32), np.asarray(bo, np.float32)
    bv = np.asarray(bv, np.float32)
    causal_flag = bool(np.asarray(causal).item())

    nc = _get_program(causal_flag)
    in_maps = make_in_maps(X, Wq, bq, Wk, bk, Wv, Wo, causal_flag)
    res = run_bass_kernel_spmd(nc, in_maps, core_ids=list(range(8)))

    # attn rows sum to 1, so the missing V bias contributes bv @ Wo exactly
    corr = bv @ Wo + bo
    outs = []
    for b in range(4):
        o = res.results[2 * b]["OUT"] + res.results[2 * b + 1]["OUT"] + corr
        outs.append(o)
    return np.stack(outs).astype(np.float32)
